# revision 21
# baseline (speedup 1.0000x reference)
"""Green's function layer kernel for Trainium2 (8 NeuronCores, data-parallel over batch).

Math: reference computes, per batch b,
    G_b = inv((w_b + i*eta) I - H_sym),  output |G_b|,
with H_sym = 0.5(H+H^T) shared across the batch and w_b a scalar from a tiny MLP.

Host eigendecomposes H_sym = Q diag(lam) Q^T once, so
    G_b = Q diag(c_b) Q^T,  c_b = 1/(w_b - lam + i*eta).

Structure exploited on top of the baseline:
 - The 32 w_b cluster within ~5*eta of each other (each is a mean over 1024
   genes), so all resonances live in one narrow eigen-window.  Batches are
   sorted by w and grouped 4-per-core; each core gets its own eigen-roll
   centering its cluster in k-block WIN, and its own mean curve
   cbar = mean_b cre_b.
 - Output tiles are [128 x 256]; each owns one PSUM bank laid out as
   [re 256 | im 256].  The bank accumulates S = Q diag(cbar) Q^T once in the
   re half (8 matmuls), then each batch adds windowed increments of BOTH the
   real delta and the imaginary part with a single packed 512-col matmul
   (rhs = [delta-scat | cim-scat]).  One ACT square then yields re^2|im^2
   together and a 256-wide DVE add forms |G|^2.  240 matmuls/core total,
   20 independent single-bank chains across all 8 PSUM banks.
 - True upper-triangle coverage at 256 cols is 20/32 tiles (0.625 of the
   matrix); the host mirrors the rest.
 - All matmuls run in bf16 (same PE rate as f32r, half the DMA/SBUF).
 - The device emits |G|^2 bf16; the host takes the sqrt, upcasts, mirrors,
   and unsorts batches.
 - A third of the output DMAs dispatch from the (idle) gpsimd sequencer:
   each dma_start costs ~0.6us of sequencer time and the sync sequencer
   alone would serialize.
"""

import numpy as np
import ml_dtypes

ETA = 0.01
B, NG, HID = 32, 1024, 64
NCORES = 8
BPC = B // NCORES  # batches per core
P = 128
KT = NG // P   # 8 k-blocks
NW2 = 256      # output tile columns (half a PSUM bank: [re 256 | im 256])
NJ4 = NG // NW2
WIN = 4                  # k-block holding every core's resonance window
CENTER = WIN * P + P // 2  # host rolls each core's cluster to this eigen-index

# Output is symmetric: keep tile (mi, J) iff mi < 2*J + 2 (covers the
# upper triangle); the rest is mirrored on the host.
KEEP = [(mi, J) for mi in range(KT) for J in range(NJ4) if mi < 2 * J + 2]
MISS = [(mi, J) for mi in range(KT) for J in range(NJ4) if mi >= 2 * J + 2]
ROW_JS = {mi: [J for J in range(NJ4) if (mi, J) in KEEP] for mi in range(KT)}

_CACHE = {}


def _build_nc():
    from concourse import bacc
    import concourse.mybir as mybir
    import concourse.tile as tile

    f32 = mybir.dt.float32
    bf16 = mybir.dt.bfloat16

    nc = bacc.Bacc("TRN2", target_bir_lowering=False, debug=False, num_devices=NCORES)

    qt_d = nc.dram_tensor("qt", [NG, NG], bf16, kind="ExternalInput").ap()
    # cc[p, 0:8] = cbar per k-block at partition p (only cols 0:8 used)
    cc_d = nc.dram_tensor("cc", [P, 16], f32, kind="ExternalInput").ap()
    # pk[p, J, :] = [cbar-scat of k-block 0, cols J | zeros] (512-col ki=0 rhs)
    pk_d = nc.dram_tensor("pk", [P, NJ4, 2 * NW2], bf16, kind="ExternalInput").ap()
    # sdi[p, b, J, :] = [delta-inc scat, cols J | cim-inc scat, cols J]
    sdi_d = nc.dram_tensor(
        "sdi", [P, BPC, NJ4, 2 * NW2], bf16, kind="ExternalInput"
    ).ap()
    out_d = nc.dram_tensor("out", [BPC, NG, NG], bf16, kind="ExternalOutput").ap()

    qt_v = qt_d.rearrange("(t p) m -> p t m", p=P)  # [128, KT, NG], k on partitions

    with tile.TileContext(nc) as tc:
        with (
            tc.tile_pool(name="qtp", bufs=1) as qtp,
            tc.tile_pool(name="scp", bufs=1) as scp,
            tc.tile_pool(name="cvp", bufs=1) as cvp,
            tc.tile_pool(name="otp", bufs=4) as otp,
            tc.tile_pool(name="psp", bufs=8, space="PSUM") as psp,
        ):
            cvec = cvp.tile([P, 16], f32, tag="cvec")
            nc.sync.dma_start(cvec[:], cc_d)
            pk = cvp.tile([P, NJ4, 2 * NW2], bf16, tag="pk")
            for c in range(2):
                cs = slice(c * 2, (c + 1) * 2)
                nc.scalar.dma_start(pk[:, cs, :], pk_d[:, cs, :])
            sdi = cvp.tile([P, BPC, NJ4, 2 * NW2], bf16, tag="sdi")
            for b in range(BPC):
                nc.scalar.dma_start(sdi[:, b, :, :], sdi_d[:, b, :, :])

            # per-k-block tiles so dependencies are fine-grained: matmuls
            # against block ki wait only for that block's load + scat.
            qt = []
            for ki in range(KT):
                qk = qtp.tile([P, NG], bf16, tag=f"qt{ki}", name=f"qt{ki}")
                CH = NG // 2
                for c in range(2):
                    cs = slice(c * CH, (c + 1) * CH)
                    nc.sync.dma_start(qk[:, cs], qt_v[:, ki, cs])
                qt.append(qk)

            # scat_c[ki][p, :] = cbar[ki*128+p] * qt[ki][p, :]  (bf16)
            scat_c = [None] * KT
            for ki in range(KT):
                sck = scp.tile([P, NG], bf16, tag=f"sc{ki}", name=f"sc{ki}")
                nc.vector.tensor_scalar_mul(sck[:], qt[ki][:], cvec[:, ki : ki + 1])
                scat_c[ki] = sck

            rd = 0
            od = 0
            for mi in range(KT):
                ms = slice(mi * P, (mi + 1) * P)
                Js = ROW_JS[mi]
                lo = Js[0] * NW2
                o = {}
                pbs = {}
                for b2 in range(BPC):
                    o[b2] = otp.tile([P, NG], bf16, tag="o", name=f"o_{mi}_{b2}")
                for J in Js:
                    pb = psp.tile([P, 2 * NW2], f32, tag="pb", name=f"pb_{mi}_{J}")
                    pbs[J] = pb
                    jc = slice(J * NW2, (J + 1) * NW2)
                    # ki=0 initializes the whole bank: [cbar-scat | zeros]
                    nc.tensor.matmul(
                        pb[:], qt[0][:, ms], pk[:, J, :], start=True, stop=False
                    )
                    for ki in range(1, KT):
                        nc.tensor.matmul(
                            pb[:, 0:NW2],
                            qt[ki][:, ms],
                            scat_c[ki][:, jc],
                            start=False,
                            stop=False,
                        )
                for b in range(BPC):
                    last = b == BPC - 1
                    for J in Js:
                        # one packed matmul updates re and im halves together
                        nc.tensor.matmul(
                            pbs[J][:],
                            qt[WIN][:, ms],
                            sdi[:, b, J, :],
                            start=False,
                            stop=last,
                        )
                    for J in Js:
                        jc = slice(J * NW2, (J + 1) * NW2)
                        s = otp.tile([P, 2 * NW2], bf16, tag="s")
                        if rd % 14 < 11:
                            nc.scalar.square(s[:], pbs[J][:])
                        else:
                            # DVE path: copy out of PSUM, square at fast rate
                            sc2 = otp.tile([P, 2 * NW2], bf16, tag="sc2")
                            nc.vector.tensor_copy(sc2[:], pbs[J][:])
                            nc.vector.tensor_mul(s[:], sc2[:], sc2[:])
                        nc.vector.tensor_add(o[b][:, jc], s[:, 0:NW2], s[:, NW2:])
                        rd += 1
                    (nc.gpsimd if od % 3 == 0 else nc.sync).dma_start(
                        out_d[b, ms, lo:NG], o[b][:, lo:NG]
                    )
                    od += 1

    nc.compile()
    return nc


def _host_prep(gene_state, H, W1, b1, W2, b2):
    # omega_net MLP -> per-batch scalar w (fp32, matching the jax reference)
    gs = gene_state.astype(np.float32).reshape(-1, HID)
    h = gs @ W1.astype(np.float32) + b1.astype(np.float32)
    h = h * (1.0 / (1.0 + np.exp(-h, dtype=np.float32)))  # SiLU
    omega = (h @ W2.astype(np.float32) + b2.astype(np.float32)).reshape(B, NG)
    w = omega.mean(axis=1).astype(np.float64)  # [B]

    Hs = 0.5 * (H.astype(np.float64) + H.astype(np.float64).T)
    lam, Q = np.linalg.eigh(Hs)  # Hs = Q diag(lam) Q^T
    qt_f32 = np.ascontiguousarray(Q.T.astype(np.float32))  # [k, n]

    bf = ml_dtypes.bfloat16
    order = np.argsort(w)  # 4 w-adjacent batches per core
    qts, ccs, pks, sdis = [], [], [], []
    for c in range(NCORES):
        bidx = order[c * BPC : (c + 1) * BPC]
        wc = w[bidx]
        r = CENTER - int(np.searchsorted(lam, wc.mean()))
        lamr = np.roll(lam, r)
        qt_c = np.roll(qt_f32, r, axis=0).astype(bf)

        d = wc[:, None] - lamr[None, :]  # [BPC, NG]
        den = d * d + ETA * ETA
        cre = d / den
        cim = -ETA / den
        cbar = cre.mean(axis=0)
        delta = cre - cbar

        cc = np.zeros((P, 16), np.float32)
        cc[:, 0:KT] = cbar.reshape(KT, P).T

        # pk: ki=0 rhs = [cbar-scat block 0 | zeros]
        sc0 = (
            cbar[0:P, None].astype(np.float32) * qt_c[0:P].astype(np.float32)
        ).astype(bf)
        pkc = np.zeros((P, NJ4, 2 * NW2), bf)
        for J in range(NJ4):
            pkc[:, J, 0:NW2] = sc0[:, J * NW2 : (J + 1) * NW2]

        # sdi: per batch, [delta-increment scat | cim-increment scat]
        win = slice(WIN * P, (WIN + 1) * P)
        qw = qt_c[win].astype(np.float32)  # [P, NG]
        sdic = np.zeros((P, BPC, NJ4, 2 * NW2), bf)
        prev_d = np.zeros(P)
        prev_i = np.zeros(P)
        for b in range(BPC):
            dinc = delta[b, win] - prev_d
            iinc = cim[b, win] - prev_i
            prev_d = delta[b, win]
            prev_i = cim[b, win]
            sd = (dinc[:, None].astype(np.float32) * qw).astype(bf)
            si = (iinc[:, None].astype(np.float32) * qw).astype(bf)
            for J in range(NJ4):
                jc = slice(J * NW2, (J + 1) * NW2)
                sdic[:, b, J, 0:NW2] = sd[:, jc]
                sdic[:, b, J, NW2:] = si[:, jc]

        qts.append(qt_c)
        ccs.append(cc)
        pks.append(pkc)
        sdis.append(sdic)
    return qts, (ccs, pks, sdis), order


def _in_maps(qts, aux, order):
    ccs, pks, sdis = aux
    return [
        {"qt": qts[c], "cc": ccs[c], "pk": pks[c], "sdi": sdis[c]}
        for c in range(NCORES)
    ]


def kernel(gene_state, H, W1, b1, W2, b2):
    from concourse.bass_utils import run_bass_kernel_spmd

    qts, aux, order = _host_prep(gene_state, H, W1, b1, W2, b2)

    if "nc" not in _CACHE:
        _CACHE["nc"] = _build_nc()
    nc = _CACHE["nc"]

    res = run_bass_kernel_spmd(
        nc, _in_maps(qts, aux, order), core_ids=list(range(NCORES))
    )
    g2 = np.concatenate(
        [np.asarray(r["out"], dtype=np.float32) for r in res.results], axis=0
    )
    # Mirror the skipped lower-triangle tiles from the computed upper ones.
    for mi, J in MISS:
        r0, r1 = mi * P, (mi + 1) * P
        c0, c1 = J * NW2, (J + 1) * NW2
        g2[:, r0:r1, c0:c1] = g2[:, c0:c1, r0:r1].swapaxes(1, 2)
    out = np.sqrt(g2)
    # Unsort: core c, slot b computed original batch order[c*BPC+b].
    full = np.empty_like(out)
    full[np.asarray(order)] = out
    return full


# revision 22
# speedup vs baseline: 1.1782x; 1.1782x over previous
"""Green's function layer kernel for Trainium2 (8 NeuronCores, data-parallel over batch).

Math: reference computes, per batch b,
    G_b = inv((w_b + i*eta) I - H_sym),  output |G_b|,
with H_sym = 0.5(H+H^T) shared across the batch and w_b a scalar from a tiny MLP.

Host eigendecomposes H_sym = Q diag(lam) Q^T once, so
    G_b = Q diag(c_b) Q^T,  c_b = 1/(w_b - lam + i*eta).

Structure exploited on top of the baseline:
 - The 32 w_b cluster within ~5*eta of each other (each is a mean over 1024
   genes), so all resonances live in one narrow eigen-window.  Batches are
   sorted by w and grouped 4-per-core; each core gets its own eigen-roll
   centering its cluster in k-block WIN, and its own mean curve
   cbar = mean_b cre_b.
 - Per output tile, PSUM accumulates S = Q diag(cbar) Q^T once (8 matmuls),
   then per batch only the *increment* diag(delta_b - delta_{b-1}) restricted
   to the window block (1 matmul) is added in place.  The imaginary part is
   rank-128, computed fresh per batch (1 matmul) in rotating banks.
   480 -> 192 matmuls/core.
 - All matmuls run in bf16 (same PE rate as f32r, half the DMA/SBUF).
 - The device emits |G|^2 = re^2 + im^2 in bf16; the host takes the sqrt,
   upcasts, mirrors the symmetric lower-triangle tiles, and unsorts batches.
 - A third of the output DMAs dispatch from the (idle) gpsimd sequencer:
   each dma_start costs ~0.6us of sequencer time and the sync sequencer
   alone would serialize.
"""

import numpy as np
import ml_dtypes

ETA = 0.01
B, NG, HID = 32, 1024, 64
NCORES = 8
BPC = B // NCORES  # batches per core
P = 128
KT = NG // P   # 8 k-blocks
NW = 512       # one fp32 PSUM bank of matmul moving free dim
NJ2 = NG // NW
WIN = 4                  # k-block holding every core's resonance window
CENTER = WIN * P + P // 2  # host rolls each core's cluster to this eigen-index

# Output is symmetric: keep tile (mi, J) iff mi < 4*J + 4 (covers the
# upper triangle); the rest is mirrored on the host.
KEEP = [(mi, J) for mi in range(KT) for J in range(NJ2) if mi < 4 * J + 4]
MISS = [(mi, J) for mi in range(KT) for J in range(NJ2) if mi >= 4 * J + 4]

_CACHE = {}


def _build_nc():
    from concourse import bacc
    import concourse.mybir as mybir
    import concourse.tile as tile

    f32 = mybir.dt.float32
    bf16 = mybir.dt.bfloat16

    nc = bacc.Bacc("TRN2", target_bir_lowering=False, debug=False, num_devices=NCORES)

    qt_d = nc.dram_tensor("qt", [NG, NG], bf16, kind="ExternalInput").ap()
    # cc[p, 0:8]  = cbar per k-block at partition p
    # cc[p, 8:12] = windowed delta-re increments (4 batches)
    # cc[p, 12:16]= windowed cim values (4 batches)
    cc_d = nc.dram_tensor("cc", [P, 16], f32, kind="ExternalInput").ap()
    out_d = nc.dram_tensor("out", [BPC, NG, NG], bf16, kind="ExternalOutput").ap()

    qt_v = qt_d.rearrange("(t p) m -> p t m", p=P)  # [128, KT, NG], k on partitions

    with tile.TileContext(nc) as tc:
        with (
            tc.tile_pool(name="qtp", bufs=1) as qtp,
            tc.tile_pool(name="scp", bufs=1) as scp,
            tc.tile_pool(name="cvp", bufs=1) as cvp,
            tc.tile_pool(name="otp", bufs=6) as otp,
            tc.tile_pool(name="pspr", bufs=1, space="PSUM") as pspr,
            tc.tile_pool(name="pspi", bufs=3, space="PSUM") as pspi,
        ):
            cvec = cvp.tile([P, 16], f32, tag="cvec")
            nc.sync.dma_start(cvec[:], cc_d)

            # per-k-block tiles so dependencies are fine-grained: matmuls
            # against block ki wait only for that block's load + scat.
            qt = []
            for ki in range(KT):
                qk = qtp.tile([P, NG], bf16, tag=f"qt{ki}", name=f"qt{ki}")
                CH = NG // 2
                for c in range(2):
                    cs = slice(c * CH, (c + 1) * CH)
                    nc.sync.dma_start(qk[:, cs], qt_v[:, ki, cs])
                qt.append(qk)

            # scaled copies of Q^T rows (all bf16):
            #   scat_c[ki][p, :] = cbar[ki*128+p] * qt[ki][p, :]
            #   scat_d[p, b, :]  = dinc_b[p]      * qt[WIN][p, :]
            #   scat_i[p, b, :]  = cim_b[p]       * qt[WIN][p, :]
            scat_c = [None] * KT
            scat_d = scp.tile([P, BPC, NG], bf16, tag="sd")
            scat_i = scp.tile([P, BPC, NG], bf16, tag="si")

            def make_scat_c(ki):
                sck = scp.tile([P, NG], bf16, tag=f"sc{ki}", name=f"sc{ki}")
                nc.vector.tensor_scalar_mul(sck[:], qt[ki][:], cvec[:, ki : ki + 1])
                scat_c[ki] = sck

            for ki in range(WIN + 1):
                make_scat_c(ki)
            nc.vector.tensor_scalar_mul(scat_d[:, 0, :], qt[WIN][:], cvec[:, 8:9])
            nc.vector.tensor_scalar_mul(scat_i[:, 0, :], qt[WIN][:], cvec[:, 12:13])
            for ki in range(WIN + 1, KT):
                make_scat_c(ki)
            for b in range(1, BPC):
                nc.vector.tensor_scalar_mul(
                    scat_d[:, b, :], qt[WIN][:], cvec[:, 8 + b : 9 + b]
                )
                nc.vector.tensor_scalar_mul(
                    scat_i[:, b, :], qt[WIN][:], cvec[:, 12 + b : 13 + b]
                )

            rd = 0
            od = 0
            for mi in range(KT):
                ms = slice(mi * P, (mi + 1) * P)
                Js = [J for J in range(NJ2) if (mi, J) in KEEP]
                psr = {
                    J: pspr.tile(
                        [P, NW], f32, tag=f"psr{J}", name=f"psr{J}_{mi}", bufs=2
                    )
                    for J in Js
                }
                # S = Q diag(cbar) Q^T accumulated once per tile
                for ki in range(KT):
                    for J in Js:
                        js = slice(J * NW, (J + 1) * NW)
                        nc.tensor.matmul(
                            psr[J][:],
                            qt[ki][:, ms],
                            scat_c[ki][:, js],
                            start=(ki == 0),
                            stop=False,
                        )
                # per-batch increments + readout
                for b in range(BPC):
                    last = b == BPC - 1
                    pis = {}
                    for J in Js:
                        js = slice(J * NW, (J + 1) * NW)
                        nc.tensor.matmul(
                            psr[J][:],
                            qt[WIN][:, ms],
                            scat_d[:, b, js],
                            start=False,
                            stop=last,
                        )
                        pi = pspi.tile(
                            [P, NW], f32, tag=f"psi{J}", name=f"pi_{mi}_{b}{J}",
                            bufs=2,
                        )
                        nc.tensor.matmul(
                            pi[:], qt[WIN][:, ms], scat_i[:, b, js],
                            start=True, stop=True,
                        )
                        pis[J] = pi
                    for J in Js:
                        js = slice(J * NW, (J + 1) * NW)
                        s1 = otp.tile([P, NW], bf16, tag="s1")
                        nc.scalar.square(s1[:], psr[J][:])
                        s2 = otp.tile([P, NW], bf16, tag="s2")
                        if rd % 2 == 0:
                            nc.scalar.square(s2[:], pis[J][:])
                        else:
                            # DVE cannot read two PSUM operands: copy out
                            # (casting to bf16), then square at the fast rate.
                            s2c = otp.tile([P, NW], bf16, tag="s2c")
                            nc.vector.tensor_copy(s2c[:], pis[J][:])
                            nc.vector.tensor_mul(s2[:], s2c[:], s2c[:])
                        o = otp.tile([P, NW], bf16, tag="o")
                        nc.vector.tensor_add(o[:], s1[:], s2[:])
                        (nc.gpsimd if od % 2 == 0 else nc.sync).dma_start(
                            out_d[b, ms, js], o[:]
                        )
                        od += 1
                        rd += 1

    nc.compile()
    return nc


def _host_prep(gene_state, H, W1, b1, W2, b2):
    # omega_net MLP -> per-batch scalar w (fp32, matching the jax reference)
    gs = gene_state.astype(np.float32).reshape(-1, HID)
    h = gs @ W1.astype(np.float32) + b1.astype(np.float32)
    h = h * (1.0 / (1.0 + np.exp(-h, dtype=np.float32)))  # SiLU
    omega = (h @ W2.astype(np.float32) + b2.astype(np.float32)).reshape(B, NG)
    w = omega.mean(axis=1).astype(np.float64)  # [B]

    Hs = 0.5 * (H.astype(np.float64) + H.astype(np.float64).T)
    lam, Q = np.linalg.eigh(Hs)  # Hs = Q diag(lam) Q^T
    qt_f32 = np.ascontiguousarray(Q.T.astype(np.float32))  # [k, n]

    order = np.argsort(w)  # 4 w-adjacent batches per core
    qts, ccs = [], []
    for c in range(NCORES):
        bidx = order[c * BPC : (c + 1) * BPC]
        wc = w[bidx]
        r = CENTER - int(np.searchsorted(lam, wc.mean()))
        lamr = np.roll(lam, r)
        qt_c = np.roll(qt_f32, r, axis=0).astype(ml_dtypes.bfloat16)

        d = wc[:, None] - lamr[None, :]  # [BPC, NG]
        den = d * d + ETA * ETA
        cre = d / den
        cim = -ETA / den
        cbar = cre.mean(axis=0)
        delta = cre - cbar

        cc = np.zeros((P, 16), np.float32)
        cc[:, 0:KT] = cbar.reshape(KT, P).T
        win = slice(WIN * P, (WIN + 1) * P)
        prev_d = np.zeros(P)
        for b in range(BPC):
            cc[:, 8 + b] = delta[b, win] - prev_d
            cc[:, 12 + b] = cim[b, win]
            prev_d = delta[b, win]
        qts.append(qt_c)
        ccs.append(cc)
    return qts, ccs, order


def _in_maps(qts, ccs, order):
    return [{"qt": qts[c], "cc": ccs[c]} for c in range(NCORES)]


def kernel(gene_state, H, W1, b1, W2, b2):
    from concourse.bass_utils import run_bass_kernel_spmd

    qts, ccs, order = _host_prep(gene_state, H, W1, b1, W2, b2)

    if "nc" not in _CACHE:
        _CACHE["nc"] = _build_nc()
    nc = _CACHE["nc"]

    res = run_bass_kernel_spmd(
        nc, _in_maps(qts, ccs, order), core_ids=list(range(NCORES))
    )
    g2 = np.concatenate(
        [np.asarray(r["out"], dtype=np.float32) for r in res.results], axis=0
    )
    # Mirror the skipped lower-triangle tiles from the computed upper ones.
    for mi, J in MISS:
        r0, r1 = mi * P, (mi + 1) * P
        c0, c1 = J * NW, (J + 1) * NW
        g2[:, r0:r1, c0:c1] = g2[:, c0:c1, r0:r1].swapaxes(1, 2)
    out = np.sqrt(g2)
    # Unsort: core c, slot b computed original batch order[c*BPC+b].
    full = np.empty_like(out)
    full[np.asarray(order)] = out
    return full


# revision 23
# speedup vs baseline: 1.1916x; 1.0114x over previous
"""Green's function layer kernel for Trainium2 (8 NeuronCores, data-parallel over batch).

Math: reference computes, per batch b,
    G_b = inv((w_b + i*eta) I - H_sym),  output |G_b|,
with H_sym = 0.5(H+H^T) shared across the batch and w_b a scalar from a tiny MLP.

Host eigendecomposes H_sym = Q diag(lam) Q^T once, so
    G_b = Q diag(c_b) Q^T,  c_b = 1/(w_b - lam + i*eta).

Structure exploited on top of the baseline:
 - The 32 w_b cluster within ~5*eta of each other (each is a mean over 1024
   genes), so all resonances live in one narrow eigen-window.  Batches are
   sorted by w and grouped 4-per-core; each core gets its own eigen-roll
   centering its cluster in k-block WIN, and its own mean curve
   cbar = mean_b cre_b.
 - Per output tile, PSUM accumulates S = Q diag(cbar) Q^T once (8 matmuls),
   then per batch only the *increment* diag(delta_b - delta_{b-1}) restricted
   to the window block (1 matmul) is added in place.  The imaginary part is
   rank-128, computed fresh per batch (1 matmul) in rotating banks.
   480 -> 192 matmuls/core.
 - All matmuls run in bf16 (same PE rate as f32r, half the DMA/SBUF).
 - The device emits |G|^2 = re^2 + im^2 in bf16; the host takes the sqrt,
   upcasts, mirrors the symmetric lower-triangle tiles, and unsorts batches.
 - A third of the output DMAs dispatch from the (idle) gpsimd sequencer:
   each dma_start costs ~0.6us of sequencer time and the sync sequencer
   alone would serialize.
"""

import numpy as np
import ml_dtypes

ETA = 0.01
B, NG, HID = 32, 1024, 64
NCORES = 8
BPC = B // NCORES  # batches per core
P = 128
KT = NG // P   # 8 k-blocks
NW = 512       # one fp32 PSUM bank of matmul moving free dim
NJ2 = NG // NW
WIN = 4                  # k-block holding every core's resonance window
CENTER = WIN * P + P // 2  # host rolls each core's cluster to this eigen-index

# Output is symmetric: keep tile (mi, J) iff mi < 4*J + 4 (covers the
# upper triangle); the rest is mirrored on the host.
KEEP = [(mi, J) for mi in range(KT) for J in range(NJ2) if mi < 4 * J + 4]
MISS = [(mi, J) for mi in range(KT) for J in range(NJ2) if mi >= 4 * J + 4]

_CACHE = {}


def _build_nc():
    from concourse import bacc
    import concourse.mybir as mybir
    import concourse.tile as tile

    f32 = mybir.dt.float32
    bf16 = mybir.dt.bfloat16

    nc = bacc.Bacc("TRN2", target_bir_lowering=False, debug=False, num_devices=NCORES)

    qt_d = nc.dram_tensor("qt", [NG, NG], bf16, kind="ExternalInput").ap()
    # cc[p, 0:8]  = cbar per k-block at partition p
    # cc[p, 8:12] = windowed delta-re increments (4 batches)
    # cc[p, 12:16]= windowed cim values (4 batches)
    cc_d = nc.dram_tensor("cc", [P, 16], f32, kind="ExternalInput").ap()
    out_d = nc.dram_tensor("out", [BPC, NG, NG], bf16, kind="ExternalOutput").ap()

    qt_v = qt_d.rearrange("(t p) m -> p t m", p=P)  # [128, KT, NG], k on partitions

    with tile.TileContext(nc) as tc:
        with (
            tc.tile_pool(name="qtp", bufs=1) as qtp,
            tc.tile_pool(name="scp", bufs=1) as scp,
            tc.tile_pool(name="cvp", bufs=1) as cvp,
            tc.tile_pool(name="otp", bufs=4) as otp,
            tc.tile_pool(name="pspr", bufs=1, space="PSUM") as pspr,
            tc.tile_pool(name="pspi", bufs=3, space="PSUM") as pspi,
        ):
            cvec = cvp.tile([P, 16], f32, tag="cvec")
            nc.sync.dma_start(cvec[:], cc_d)

            # per-k-block tiles so dependencies are fine-grained: matmuls
            # against block ki wait only for that block's load + scat.
            qt = []
            for ki in range(KT):
                qk = qtp.tile([P, NG], bf16, tag=f"qt{ki}", name=f"qt{ki}")
                CH = NG // 2
                for c in range(2):
                    cs = slice(c * CH, (c + 1) * CH)
                    nc.sync.dma_start(qk[:, cs], qt_v[:, ki, cs])
                qt.append(qk)

            # scaled copies of Q^T rows (all bf16):
            #   scat_c[ki][p, :] = cbar[ki*128+p] * qt[ki][p, :]
            #   scat_d[p, b, :]  = dinc_b[p]      * qt[WIN][p, :]
            #   scat_i[p, b, :]  = cim_b[p]       * qt[WIN][p, :]
            scat_c = [None] * KT
            scat_d = scp.tile([P, BPC, NG], bf16, tag="sd")
            scat_i = scp.tile([P, BPC, NG], bf16, tag="si")

            def make_scat_c(ki):
                sck = scp.tile([P, NG], bf16, tag=f"sc{ki}", name=f"sc{ki}")
                nc.vector.tensor_scalar_mul(sck[:], qt[ki][:], cvec[:, ki : ki + 1])
                scat_c[ki] = sck

            for ki in range(WIN + 1):
                make_scat_c(ki)
            nc.vector.tensor_scalar_mul(scat_d[:, 0, :], qt[WIN][:], cvec[:, 8:9])
            nc.vector.tensor_scalar_mul(scat_i[:, 0, :], qt[WIN][:], cvec[:, 12:13])
            for ki in range(WIN + 1, KT):
                make_scat_c(ki)
            for b in range(1, BPC):
                nc.vector.tensor_scalar_mul(
                    scat_d[:, b, :], qt[WIN][:], cvec[:, 8 + b : 9 + b]
                )
                nc.vector.tensor_scalar_mul(
                    scat_i[:, b, :], qt[WIN][:], cvec[:, 12 + b : 13 + b]
                )

            rd = 0
            od = 0
            for mi in range(KT):
                ms = slice(mi * P, (mi + 1) * P)
                Js = [J for J in range(NJ2) if (mi, J) in KEEP]
                psr = {
                    J: pspr.tile(
                        [P, NW], f32, tag=f"psr{J}", name=f"psr{J}_{mi}", bufs=2
                    )
                    for J in Js
                }
                # S = Q diag(cbar) Q^T accumulated once per tile
                for ki in range(KT):
                    for J in Js:
                        js = slice(J * NW, (J + 1) * NW)
                        nc.tensor.matmul(
                            psr[J][:],
                            qt[ki][:, ms],
                            scat_c[ki][:, js],
                            start=(ki == 0),
                            stop=False,
                        )
                # per-batch increments + readout
                for b in range(BPC):
                    last = b == BPC - 1
                    pis = {}
                    for J in Js:
                        js = slice(J * NW, (J + 1) * NW)
                        nc.tensor.matmul(
                            psr[J][:],
                            qt[WIN][:, ms],
                            scat_d[:, b, js],
                            start=False,
                            stop=last,
                        )
                        pi = pspi.tile(
                            [P, NW], f32, tag=f"psi{J}", name=f"pi_{mi}_{b}{J}",
                            bufs=2,
                        )
                        nc.tensor.matmul(
                            pi[:], qt[WIN][:, ms], scat_i[:, b, js],
                            start=True, stop=True,
                        )
                        pis[J] = pi
                    for J in Js:
                        js = slice(J * NW, (J + 1) * NW)
                        s1 = otp.tile([P, NW], bf16, tag="s1")
                        nc.scalar.square(s1[:], psr[J][:])
                        s2 = otp.tile([P, NW], bf16, tag="s2")
                        if rd % 2 == 0:
                            nc.scalar.square(s2[:], pis[J][:])
                        else:
                            # DVE cannot read two PSUM operands: copy out
                            # (casting to bf16), then square at the fast rate.
                            s2c = otp.tile([P, NW], bf16, tag="s2c")
                            nc.vector.tensor_copy(s2c[:], pis[J][:])
                            nc.vector.tensor_mul(s2[:], s2c[:], s2c[:])
                        o = otp.tile([P, NW], bf16, tag="o")
                        nc.vector.tensor_add(o[:], s1[:], s2[:])
                        (nc.gpsimd if od % 3 == 0 else nc.sync).dma_start(
                            out_d[b, ms, js], o[:]
                        )
                        od += 1
                        rd += 1

    nc.compile()
    return nc


def _host_prep(gene_state, H, W1, b1, W2, b2):
    # omega_net MLP -> per-batch scalar w (fp32, matching the jax reference)
    gs = gene_state.astype(np.float32).reshape(-1, HID)
    h = gs @ W1.astype(np.float32) + b1.astype(np.float32)
    h = h * (1.0 / (1.0 + np.exp(-h, dtype=np.float32)))  # SiLU
    omega = (h @ W2.astype(np.float32) + b2.astype(np.float32)).reshape(B, NG)
    w = omega.mean(axis=1).astype(np.float64)  # [B]

    Hs = 0.5 * (H.astype(np.float64) + H.astype(np.float64).T)
    lam, Q = np.linalg.eigh(Hs)  # Hs = Q diag(lam) Q^T
    qt_f32 = np.ascontiguousarray(Q.T.astype(np.float32))  # [k, n]

    order = np.argsort(w)  # 4 w-adjacent batches per core
    qts, ccs = [], []
    for c in range(NCORES):
        bidx = order[c * BPC : (c + 1) * BPC]
        wc = w[bidx]
        r = CENTER - int(np.searchsorted(lam, wc.mean()))
        lamr = np.roll(lam, r)
        qt_c = np.roll(qt_f32, r, axis=0).astype(ml_dtypes.bfloat16)

        d = wc[:, None] - lamr[None, :]  # [BPC, NG]
        den = d * d + ETA * ETA
        cre = d / den
        cim = -ETA / den
        cbar = cre.mean(axis=0)
        delta = cre - cbar

        cc = np.zeros((P, 16), np.float32)
        cc[:, 0:KT] = cbar.reshape(KT, P).T
        win = slice(WIN * P, (WIN + 1) * P)
        prev_d = np.zeros(P)
        for b in range(BPC):
            cc[:, 8 + b] = delta[b, win] - prev_d
            cc[:, 12 + b] = cim[b, win]
            prev_d = delta[b, win]
        qts.append(qt_c)
        ccs.append(cc)
    return qts, ccs, order


def _in_maps(qts, ccs, order):
    return [{"qt": qts[c], "cc": ccs[c]} for c in range(NCORES)]


def kernel(gene_state, H, W1, b1, W2, b2):
    from concourse.bass_utils import run_bass_kernel_spmd

    qts, ccs, order = _host_prep(gene_state, H, W1, b1, W2, b2)

    if "nc" not in _CACHE:
        _CACHE["nc"] = _build_nc()
    nc = _CACHE["nc"]

    res = run_bass_kernel_spmd(
        nc, _in_maps(qts, ccs, order), core_ids=list(range(NCORES))
    )
    g2 = np.concatenate(
        [np.asarray(r["out"], dtype=np.float32) for r in res.results], axis=0
    )
    # Mirror the skipped lower-triangle tiles from the computed upper ones.
    for mi, J in MISS:
        r0, r1 = mi * P, (mi + 1) * P
        c0, c1 = J * NW, (J + 1) * NW
        g2[:, r0:r1, c0:c1] = g2[:, c0:c1, r0:r1].swapaxes(1, 2)
    out = np.sqrt(g2)
    # Unsort: core c, slot b computed original batch order[c*BPC+b].
    full = np.empty_like(out)
    full[np.asarray(order)] = out
    return full
